# revision 40
# baseline (speedup 1.0000x reference)
"""Trainium2 Bass kernel for nn_AttentionOnDetail (sparse patch attention).

Data-parallel over batch B=8 across 8 NeuronCores; one batch per core.
v3 redesign (latency-focused; the kernel is dependency-bound):
  - x streamed first (x tile DMAs are the first SP descriptors; tile 3 in
    four 512-col chunks).  pw row + f32 ident ride gpsimd SWDGE so their
    transfers slip into the stream right after tile 0.
  - pw broadcast stays in PSUM (dot stt reads PSUM directly, no copies);
    PE warmup matmuls run on the eps tile at t~0.9 so everything after
    runs at peak p-state.
  - stats split across engines: ACT squares (t0-t2, 3a, 3b, 3d), DVE dots
    (t0-t2, 3c, 3d) + logit chain, Pool dots (3a, 3b) + square (3c).
  - patch logits use the monotone transform dot*|dot|/ms (no Sqrt); the
    selection runs once globally: top8 -> threshold mask * negio -> top8
    gives the ranks directly (idc matmul folds NP-v and the *16).
  - single ACT function set (Exp/Square/Copy, set 0) loaded once at t=0;
    rmsnorm rsqrt = Newton iteration on Pool (bit-trick seed), sigmoid
    via exp(-g), softmax exp with folded -6 bias so p fits fp16.
  - everything from the projection on runs in fp16 (W cast on host).
  - attention computed transposed (att_T = k^T q) with the causal mask
    preloaded into PSUM via an identity matmul; denominator broadcast to
    all partitions by a single ones[65,128] matmul; gate folded as
    yg = y / ((1+e^-g)*den) with DVE/Pool divide; output projection
    consumes yg^T directly.
"""

import sys
import numpy as np

for _p in ("/opt/trn_rl_repo",):
    if _p not in sys.path:
        sys.path.insert(0, _p)

import concourse.bass as bass
import concourse.bacc as bacc
import concourse.tile as tile
from concourse import mybir
from concourse.bass_utils import run_bass_kernel_spmd

F32 = mybir.dt.float32
F32R = mybir.dt.float32r
F16 = mybir.dt.float16
I32 = mybir.dt.int32
U32 = mybir.dt.uint32
AF = mybir.ActivationFunctionType
ALU = mybir.AluOpType
AX = mybir.AxisListType

B, T, C, H, T0 = 8, 8192, 128, 8, 16
NP = T // T0          # 512 patches
PATCH = T0 * C        # 2048 elements per patch
S = 65                # sink + 64 selected tokens
NSEL = 64
EPS = 1.1920929e-07
SCALE = 1.0 / float(np.sqrt(np.float32(C)))
EXPB = -6.0           # softmax exp bias; den-normalization cancels it
NEG_BIG = -60000.0    # additive causal mask (fp16-representable)
MAGIC = 0x5F3759DF    # fast-rsqrt seed

# tabs2 f32-column layout (fp16 payloads packed as pairs into f32 cols)
TB_SEL16 = 0          # sel16' f32 [5, 64]
TB_IDF16 = 64         # ident f16 [128, 128] -> 64 f32 cols
TB_COS = 128          # cosdup f16 [128, 128] -> 64
TB_SIN = 192          # sinpm f16 [128, 128] -> 64
TB_SINKQ = 256        # sinkTq f16 [128, 8] -> 4
TB_SINKK = 260        # sinkTk f16 [128, 8] -> 4
TB_CMASK = 264        # cmaskT f16 [65, 66] -> 33 (col 65 pad)
TB_NEGIO = 297        # negio f16 [1, 512] -> 256
TB_COLS = 553

# PSUM f32-column region plan (8 banks x 512 cols)
PB_PWB = 0            # pwB broadcast [128, 2048] (cols 0:2048), early only
PB_QNT16 = 0          # qkn transposes (f16 cols 0:1024 = f32 0:512)
PB_GT16 = 2048        # gT transposes (f16 cols 2048:2560 = f32 1024:1280)
PB_YT = 1536          # yT [128, (h,s)=512] cols 1536:2048 (bank 3)
PB_OUT = 3584         # out [64, 128] in bank 7 (logits row dead)
PB_ATT0 = 2048        # att_T group 0 [65, 260]
PB_ATT1 = 2560        # att_T group 1 [65, 260]
PB_XSELT = 3072       # x_selT staging [128, 64]
PB_DENB = 3072        # den broadcast [128, 512] (after x_selT dead)
PB_WARM = 3500        # warmup scratch col
PB_MM8 = 3582         # mm8 transpose column [4, 1]
PB_IDC = 3583         # sel16 matmul out [64, 1]
LROW = 3584           # logits row [1, 512] (rows 0:1)


def rap(t, apl, offset=0):
    """Raw AP over a tile/AP's storage, flat element strides."""
    base = t if isinstance(t, bass.AP) else t[:]
    return bass.AP(tensor=base.tensor, offset=base.offset + offset,
                   ap=[list(x) for x in apl])


def build_kernel(nc):
    xb = nc.dram_tensor("xb", [T, C], F32, kind="ExternalInput")
    pw = nc.dram_tensor("pw", [1, PATCH + 128], F32R, kind="ExternalInput")
    identd = nc.dram_tensor("identd", [128, 128], F32, kind="ExternalInput")
    tabs2 = nc.dram_tensor("tabs2", [128, TB_COLS], F32, kind="ExternalInput")
    wqT_d = nc.dram_tensor("wqT_d", [C, 4 * C * H], F16, kind="ExternalInput")
    woT_d = nc.dram_tensor("woT_d", [C, H, C], F16, kind="ExternalInput")
    sinkv = nc.dram_tensor("sinkv", [1, H * C], F16, kind="ExternalInput")
    out = nc.dram_tensor("out", [NSEL, C], F32, kind="ExternalOutput")

    with tile.TileContext(nc) as tc:
        _emit(tc, nc, xb, pw, identd, tabs2, wqT_d, woT_d, sinkv, out)
    return nc


def _emit(tc, nc, xb, pw, identd, tabs2_d, wqT_d, woT_d, sinkv, out):
    import os
    LEVEL = int(os.environ.get("KLEVEL", "9"))
    from contextlib import ExitStack
    ctx = ExitStack()
    with ctx:
        const1 = ctx.enter_context(tc.tile_pool(name="const1", bufs=1))
        xpool = ctx.enter_context(tc.tile_pool(name="xpool", bufs=1))
        junkp = ctx.enter_context(tc.tile_pool(name="junkp", bufs=1))
        stat = ctx.enter_context(tc.tile_pool(name="stat", bufs=4))
        sb = ctx.enter_context(tc.tile_pool(name="sb", bufs=1))
        psall = ctx.enter_context(tc.tile_pool(name="psall", bufs=1,
                                               space="PSUM"))
        PS = psall.tile([128, 4096], F32)
        PS16 = PS[:, :].bitcast(F16)  # [128, 8192] f16 view

        # ---------------- x stream first; pw/ident via gpsimd --------------
        def xdma(i):
            xp = xpool.tile([128, PATCH], F32, tag=f"xp{i}")
            nc.sync.dma_start(
                out=xp[:, :],
                in_=rap(xb[:, :], [[PATCH, 128], [1, PATCH]],
                        offset=i * 128 * PATCH))
            return xp

        xps = [xdma(0)]
        pwo_sb = const1.tile([1, PATCH + 128], F32R)
        nc.gpsimd.dma_start(out=pwo_sb[:, :], in_=pw[:, :])
        identf = const1.tile([128, 128], F32)
        nc.gpsimd.dma_start(out=identf[:, :], in_=identd[:, :])
        xps.append(xdma(1))
        xps.append(xdma(2))
        xp3 = xpool.tile([128, PATCH], F32, tag="xp3")
        for ch in range(4):
            nc.sync.dma_start(
                out=xp3[:, 512 * ch:512 * (ch + 1)],
                in_=rap(xb[:, :], [[PATCH, 128], [1, 512]],
                        offset=3 * 128 * PATCH + 512 * ch))
        xps.append(xp3)

        tabs2 = const1.tile([128, TB_COLS], F32)
        nc.sync.dma_start(out=tabs2[:, :], in_=tabs2_d[:, :])
        t2h = tabs2[:, :].bitcast(F16)  # [128, 2*TB_COLS] f16 view

        def h16(col_f32, ncols_f16, nrows=128):
            return rap(t2h, [[2 * TB_COLS, nrows], [1, ncols_f16]],
                       offset=2 * col_f32)

        identh_v = h16(TB_IDF16, 128)
        identh = None  # materialized below after tabs2 lands
        cosdup = h16(TB_COS, 128)
        sinkTq = h16(TB_SINKQ, 8)
        sinkTk = h16(TB_SINKK, 8)
        negio = h16(TB_NEGIO, 512, nrows=1)
        sel16 = tabs2[0:5, TB_SEL16:TB_SEL16 + 64]

        wqT = const1.tile([C, 4 * C * H], F16)
        for wch in range(4):
            nc.sync.dma_start(out=wqT[:, 1024 * wch:1024 * (wch + 1)],
                              in_=wqT_d[:, 1024 * wch:1024 * (wch + 1)])
        woTb = const1.tile([C, H, C], F16)
        nc.sync.dma_start(out=woTb[:, :, :], in_=woT_d[:, :, :])
        v_sb = sb.tile([S, H, C], F16, tag="v_sb")
        nc.sync.dma_start(
            out=v_sb[NSEL:S, :, :],
            in_=sinkv[:, :].rearrange("p (h c) -> p h c", h=H))

        identh_t = const1.tile([128, 128], F16)
        nc.scalar.copy(out=identh_t[:, :], in_=identh_v)
        identh = identh_t[:, :]
        eps_t = const1.tile([128, 1], F32)
        nc.vector.memset(eps_t[:, :], EPS)
        expb_t = const1.tile([S, 1], F32)
        nc.vector.memset(expb_t[:, :], EXPB)
        ones65B = const1.tile([S, C], F16)
        nc.vector.memset(ones65B[:, :], 1.0)
        rhs5 = const1.tile([5, 1], F32)
        nc.vector.memset(rhs5[:, :], 1.0)
        # preload ACT set 0 (Exp/Square/Copy) once, while DMAs stream
        dummy = stat.tile([1, 1], F32)
        nc.vector.memset(dummy[:, :], 1.0)
        nc.scalar.activation(out=dummy[:, :], in_=dummy[:, :], func=AF.Exp)

        # PE warmups on the eps tile (lift p-state early)
        nc.tensor.matmul(out=PS[0:1, PB_WARM:PB_WARM + 1], lhsT=eps_t[:, :],
                         rhs=eps_t[:, :], start=True, stop=True)
        nc.tensor.matmul(out=PS[0:1, PB_WARM:PB_WARM + 1], lhsT=eps_t[:, :],
                         rhs=eps_t[:, :], start=True, stop=True)
        # pwB broadcast via K=1 matmuls; dots read it from PSUM directly
        ones_t = pwo_sb[0:1, PATCH:PATCH + 128]
        for q4 in range(4):
            nc.tensor.matmul(out=PS[:, PB_PWB + 512 * q4:
                                    PB_PWB + 512 * (q4 + 1)],
                             lhsT=ones_t,
                             rhs=pwo_sb[0:1, 512 * q4:512 * (q4 + 1)],
                             start=True, stop=True)

        # ---------------- phase 1: per-patch stats ----------------
        junk = junkp.tile([128, PATCH], F32, tag="junk")
        junk2 = junkp.tile([128, PATCH], F32, tag="junk2")
        junk3 = junkp.tile([128, PATCH], F32, tag="junk3")
        junk23 = junkp.tile([128, PATCH], F32, tag="junk23")
        ss_c = stat.tile([128, 4], F32, tag="ss_c")
        dot_c = stat.tile([128, 4], F32, tag="dot_c")
        ss3 = stat.tile([128, 4], F32, tag="ss3")
        dot3 = stat.tile([128, 4], F32, tag="dot3")
        msx = stat.tile([128, 4], F32, tag="msx")
        dd = stat.tile([128, 4], F32, tag="dd")
        logit_c = stat.tile([128, 4], F32, tag="logit_c")

        nd = stat.tile([128, 4], F32, tag="nd")
        rms = stat.tile([128, 4], F32, tag="rms")

        def logit_tile(i, eng):
            # ms = ss/PATCH + EPS; logit' = dot*|dot| * recip(ms) (order-eq)
            eng.tensor_scalar(
                out=msx[:, i:i + 1], in0=ss_c[:, i:i + 1],
                scalar1=1.0 / PATCH, scalar2=EPS, op0=ALU.mult, op1=ALU.add)
            nc.vector.reciprocal(out=rms[:, i:i + 1], in_=msx[:, i:i + 1])
            nc.vector.tensor_scalar(
                out=nd[:, i:i + 1].bitcast(I32),
                in0=dot_c[:, i:i + 1].bitcast(I32),
                scalar1=0x7FFFFFFF, scalar2=None, op0=ALU.bitwise_and)
            eng.tensor_tensor(
                out=dd[:, i:i + 1], in0=nd[:, i:i + 1],
                in1=dot_c[:, i:i + 1], op=ALU.mult)
            eng.tensor_tensor(
                out=logit_c[:, i:i + 1], in0=dd[:, i:i + 1],
                in1=rms[:, i:i + 1], op=ALU.mult)
            nc.tensor.transpose(
                out=PS[0:1, LROW + 128 * i:LROW + 128 * (i + 1)],
                in_=logit_c[:, i:i + 1], identity=identf[:, :])

        for i in range(3):
            xp = xps[i]
            nc.scalar.activation(out=junk[:, :], in_=xp[:, :],
                                 func=AF.Square,
                                 accum_out=ss_c[:, i:i + 1])
            nc.vector.scalar_tensor_tensor(
                out=junk2[:, :], in0=xp[:, :], scalar=1.0,
                in1=PS[:, PB_PWB:PB_PWB + PATCH],
                op0=ALU.mult, op1=ALU.mult,
                accum_out=dot_c[:, i:i + 1])
            logit_tile(i, nc.gpsimd)

        # tile 3 chunks: ACT squares a,b,d + Pool square c;
        # Pool dots a,b + DVE dots c,d
        def sq3(eng, ch):
            cs = slice(512 * ch, 512 * (ch + 1))
            if eng is nc.scalar:
                nc.scalar.activation(out=junk3[:, cs], in_=xp3[:, cs],
                                     func=AF.Square,
                                     accum_out=ss3[:, ch:ch + 1])
            else:
                eng.scalar_tensor_tensor(
                    out=junk3[:, cs], in0=xp3[:, cs], scalar=1.0,
                    in1=xp3[:, cs], op0=ALU.mult, op1=ALU.mult,
                    accum_out=ss3[:, ch:ch + 1])

        def dot3f(eng, ch):
            cs = slice(512 * ch, 512 * (ch + 1))
            nc.vector.scalar_tensor_tensor(
                out=junk23[:, cs], in0=xp3[:, cs], scalar=1.0,
                in1=PS[:, PB_PWB + 512 * ch:PB_PWB + 512 * (ch + 1)],
                op0=ALU.mult, op1=ALU.mult,
                accum_out=dot3[:, ch:ch + 1])

        dot3f(nc.vector, 0)
        sq3(nc.scalar, 0)
        dot3f(nc.vector, 1)
        sq3(nc.scalar, 1)
        dot3f(nc.vector, 2)
        sq3(nc.scalar, 2)
        dot3f(nc.vector, 3)
        sq3(nc.scalar, 3)
        nc.vector.tensor_reduce(out=ss_c[:, 3:4],
                                in_=ss3[:, :].rearrange("p (a f) -> p a f",
                                                        a=1),
                                axis=AX.X, op=ALU.add)
        nc.vector.tensor_reduce(out=dot_c[:, 3:4],
                                in_=dot3[:, :].rearrange("p (a f) -> p a f",
                                                         a=1),
                                axis=AX.X, op=ALU.add)
        logit_tile(3, nc.vector)

        # ---------------- top-4 selection (global, on the PSUM row) --------
        lrow = PS[0:1, LROW:LROW + NP]
        gmax8 = stat.tile([1, 8], F32, tag="gmax8")
        nc.vector.max(out=gmax8[:, :], in_=lrow)
        masked = stat.tile([1, NP], F32, tag="masked")
        nc.vector.scalar_tensor_tensor(
            out=masked[:, :], in0=lrow, scalar=gmax8[:, 3:4],
            in1=negio, op0=ALU.is_ge, op1=ALU.mult)
        mm8 = stat.tile([1, 8], F32, tag="mm8")
        nc.vector.max(out=mm8[:, :], in_=masked[:, :])

        # patch ranks (NP - v) fold into sel16'; rhs = [v0..v3, 1]
        nc.tensor.transpose(out=PS[0:4, PB_MM8:PB_MM8 + 1],
                            in_=mm8[0:1, 0:4], identity=identf[0:1, 0:1])
        nc.scalar.copy(out=rhs5[0:4, :], in_=PS[0:4, PB_MM8:PB_MM8 + 1])
        nc.tensor.matmul(out=PS[0:NSEL, PB_IDC:PB_IDC + 1], lhsT=sel16,
                         rhs=rhs5[:, :], start=True, stop=True)
        idc_i = stat.tile([NSEL, 1], I32, tag="idc_i")
        nc.vector.tensor_copy(out=idc_i[:, :],
                              in_=PS[0:NSEL, PB_IDC:PB_IDC + 1])

        if LEVEL == 1:
            l1 = stat.tile([NSEL, C], F32, tag="l1")
            nc.vector.tensor_copy(out=l1[0:4, 0:8],
                                  in_=mm8[0:1, :].to_broadcast([4, 8]))
            nc.sync.dma_start(out=out[:, :], in_=l1[:, :])
            return

        # gather the 64 tokens (row 16T+4p+t = token 16*I[p] + 4T + t)
        x_sel = sb.tile([NSEL, C], F32, tag="x_sel")
        nc.gpsimd.indirect_dma_start(
            out=x_sel[:, :], out_offset=None, in_=xb[:, :],
            in_offset=bass.IndirectOffsetOnAxis(ap=idc_i[:, 0:1], axis=0))

        if LEVEL == 2:
            nc.sync.dma_start(out=out[:, :], in_=x_sel[:, :])
            return

        # ---------------- qkvg projection (fp16) ----------------
        nc.tensor.transpose(out=PS[0:128, PB_XSELT:PB_XSELT + NSEL],
                            in_=x_sel[:, :],
                            identity=identf[0:NSEL, 0:NSEL])
        x_selT = sb.tile([C, NSEL], F16, tag="x_selT")
        nc.scalar.copy(out=x_selT[:, :], in_=PS[:, PB_XSELT:PB_XSELT + NSEL])

        for g in range(8):
            nc.tensor.matmul(out=PS[0:NSEL, 512 * g:512 * (g + 1)],
                             lhsT=x_selT[:, :],
                             rhs=wqT[:, 512 * g:512 * (g + 1)],
                             start=True, stop=True)

        # staging to fp16: qk rows 0:32 by block (ACT/DVE/Pool/ACT),
        # then vg rows 32:64
        stQK = sb.tile([32, 4 * C * H], F16, tag="stQK")
        stVG = sb.tile([32, 4 * C * H], F16, tag="stVG")
        nc.scalar.copy(out=stQK[:, 0:1024], in_=PS[0:32, 0:1024])
        nc.vector.tensor_copy(out=stQK[:, 1024:2048], in_=PS[0:32, 1024:2048])
        nc.scalar.copy(out=stQK[:, 2048:3072], in_=PS[0:32, 2048:3072])
        nc.vector.tensor_copy(out=stQK[:, 3072:4096], in_=PS[0:32, 3072:4096])
        # qk rearrange: src iterates (r, b, col) matching plain dst
        # partition order 4r+b exactly
        qk = sb.tile([128, H, C], F16, tag="qk")
        FQ = 4 * C * H
        nc.sync.dma_start(
            out=qk[:, :, :],
            in_=rap(stQK[:, :], [[FQ, 32], [1024, 4], [1, 1024]]))
        nc.scalar.copy(out=stVG[:, 0:1024], in_=PS[32:64, 0:1024])
        nc.vector.tensor_copy(out=stVG[:, 1024:2048], in_=PS[32:64, 1024:2048])
        nc.scalar.copy(out=stVG[:, 2048:3072], in_=PS[32:64, 2048:3072])
        nc.vector.tensor_copy(out=stVG[:, 3072:4096], in_=PS[32:64, 3072:4096])

        # g rearrange (rows 16:32 of stVG), then v (rows 0:16) into v_sb
        g_sb = sb.tile([NSEL, H, C], F16, tag="g_sb")
        nc.sync.dma_start(
            out=g_sb[:, :, :],
            in_=rap(stVG[:, :], [[FQ, 16], [1024, 4], [1, 1024]],
                    offset=16 * FQ))
        nc.sync.dma_start(
            out=v_sb[0:NSEL, :, :],
            in_=rap(stVG[:, :], [[FQ, 16], [1024, 4], [1, 1024]]))

        if LEVEL == 3:
            l3 = sb.tile([NSEL, C], F32, tag="l3")
            nc.vector.tensor_copy(out=l3[:, :], in_=qk[0:NSEL, 0, :])
            nc.sync.dma_start(out=out[:, :], in_=l3[:, :])
            return

        # causal-mask preload for both att groups (PE idle window)
        for g in range(2):
            attb = PB_ATT0 if g == 0 else PB_ATT1
            nc.tensor.matmul(
                out=PS[0:S, attb:attb + 4 * S],
                lhsT=identh[0:S, 0:S],
                rhs=rap(t2h, [[2 * TB_COLS, 65], [0, 4], [1, 65]],
                        offset=2 * TB_CMASK),
                start=True, stop=False)

        # ---------------- rmsnorm + rope (fp16) ----------------
        # squares: ACT heads 5:8 (accum), DVE heads 0:5 (fp16 2x + reduce)
        ssq = sb.tile([128, H], F32, tag="ssq")
        sqj = junkp.tile([128, 6, C], F16, tag="sqj")
        sqa = junkp.tile([128, 2, C], F32, tag="sqa")
        for h in range(6):
            nc.vector.scalar_tensor_tensor(
                out=sqj[:, h, :], in0=qk[:, h, :], scalar=1.0,
                in1=qk[:, h, :], op0=ALU.mult, op1=ALU.mult,
                accum_out=ssq[:, h:h + 1])
        for h in range(6, 8):
            nc.scalar.activation(out=sqa[:, h - 6, :], in_=qk[:, h, :],
                                 func=AF.Square,
                                 accum_out=ssq[:, h:h + 1])
        # rf = rsqrt(ssq/C + eps): bit-trick seed (DVE) + 2 Newton
        # steps on Pool, overlapped with rope on DVE
        msv = sb.tile([128, H], F32, tag="msv")
        nwa = sb.tile([128, H], F32, tag="nwa")
        nwb = sb.tile([128, H], F32, tag="nwb")
        yv = sb.tile([128, H], F32, tag="yv")
        rf = sb.tile([128, H], F16, tag="rf")
        nc.gpsimd.tensor_scalar(out=msv[:, :], in0=ssq[:, :],
                                scalar1=1.0 / C, scalar2=EPS,
                                op0=ALU.mult, op1=ALU.add)
        msv_i = msv[:, :].bitcast(I32)
        yv_i = yv[:, :].bitcast(I32)
        nc.vector.tensor_scalar(out=yv_i, in0=msv_i, scalar1=1,
                                scalar2=None, op0=ALU.arith_shift_right)
        nc.vector.tensor_scalar(out=yv_i, in0=yv_i, scalar1=-1,
                                scalar2=MAGIC, op0=ALU.mult, op1=ALU.add)
        for it in range(2):
            nc.gpsimd.tensor_tensor(out=nwa[:, :], in0=yv[:, :],
                                    in1=yv[:, :], op=ALU.mult)
            nc.gpsimd.tensor_tensor(out=nwb[:, :], in0=nwa[:, :],
                                    in1=msv[:, :], op=ALU.mult)
            nc.gpsimd.tensor_scalar(out=nwb[:, :], in0=nwb[:, :],
                                    scalar1=-0.5, scalar2=1.5,
                                    op0=ALU.mult, op1=ALU.add)
            nc.gpsimd.tensor_tensor(out=yv[:, :], in0=yv[:, :],
                                    in1=nwb[:, :], op=ALU.mult)
        nc.gpsimd.tensor_copy(out=rf[:, :], in_=yv[:, :])
        # rope (independent of rf): r1 = qk*cos; r2 = swap(qk)*sin
        r1 = sb.tile([128, H, C], F16, tag="r1")
        r2 = sb.tile([128, H, C], F16, tag="r2")
        qkr = sb.tile([128, H, C], F16, tag="qkr")
        qkn = sb.tile([128, H, C], F16, tag="qkn")
        nc.vector.tensor_tensor(
            out=r1[:, :, :], in0=qk[:, :, :],
            in1=cosdup.rearrange("p (a c) -> p a c", a=1)
                .to_broadcast([128, H, C]), op=ALU.mult)
        nc.vector.tensor_tensor(
            out=r2[:, :, 0:64], in0=qk[:, :, 64:128],
            in1=rap(t2h, [[2 * TB_COLS, 128], [0, H], [1, 64]],
                    offset=2 * TB_SIN),
            op=ALU.mult)
        nc.vector.tensor_tensor(
            out=r2[:, :, 64:128], in0=qk[:, :, 0:64],
            in1=rap(t2h, [[2 * TB_COLS, 128], [0, H], [1, 64]],
                    offset=2 * TB_SIN + 64),
            op=ALU.mult)
        nc.vector.tensor_add(out=qkr[:, :, :], in0=r1[:, :, :],
                             in1=r2[:, :, :])
        # qkn = qkr * rf (broadcast over c): g0 on DVE first, then g1
        nc.vector.tensor_tensor(
            out=qkn[:, 0:4, :], in0=qkr[:, 0:4, :],
            in1=rf[:, 0:4].rearrange("p (h a) -> p h a", a=1)
                .to_broadcast([128, 4, C]), op=ALU.mult)
        nc.vector.tensor_tensor(
            out=qkn[:, 4:6, :], in0=qkr[:, 4:6, :],
            in1=rf[:, 4:6].rearrange("p (h a) -> p h a", a=1)
                .to_broadcast([128, 2, C]), op=ALU.mult)
        nc.gpsimd.tensor_tensor(
            out=qkn[:, 6:8, :], in0=qkr[:, 6:8, :],
            in1=rf[:, 6:8].rearrange("p (h a) -> p h a", a=1)
                .to_broadcast([128, 2, C]), op=ALU.mult)

        if LEVEL == 4:
            l4 = sb.tile([NSEL, C], F32, tag="l4")
            nc.vector.tensor_copy(out=l4[:, :], in_=qkn[0:NSEL, 0, :])
            nc.sync.dma_start(out=out[:, :], in_=l4[:, :])
            return

        # ---------------- transposes to qnT / knT (fp16 via PSUM bitcast) --
        qnT = sb.tile([C, H, S], F16, tag="qnT")
        knT = sb.tile([C, H, S], F16, tag="knT")
        nc.scalar.copy(out=rap(qnT[:, :, :], [[H * S, C], [S, H], [1, 1]],
                               offset=NSEL),
                       in_=sinkTq.rearrange("c (h a) -> c h a", a=1))
        nc.scalar.copy(out=rap(knT[:, :, :], [[H * S, C], [S, H], [1, 1]],
                               offset=NSEL),
                       in_=sinkTk.rearrange("c (h a) -> c h a", a=1))

        if LEVEL == 41:
            l41 = sb.tile([NSEL, C], F32, tag="l41")
            nc.vector.tensor_copy(out=l41[:, 0:8],
                                  in_=qnT[0:NSEL, 0, 0:8])
            nc.vector.memset(l41[:, 8:128], 0.0)
            nc.sync.dma_start(out=out[:, :], in_=l41[:, :])
            return

        pexp = sb.tile([S, H, S], F16, tag="pexp")
        for g in range(2):
            base16 = 512 * g
            # one [128,128] transpose per head: out cols 0:64 = q^T,
            # 64:128 = k^T (tile_position (0,0); fp16 (64,0) faults)
            for j in range(4):
                h = 4 * g + j
                nc.tensor.transpose(
                    out=rap(PS16, [[8192, 128], [1, 128]],
                            offset=base16 + 128 * j),
                    in_=qkn[:, h, :],
                    identity=identh)
            for si, dstT in enumerate((qnT, knT)):
                dst = rap(dstT[:, :, :], [[H * S, C], [S, 4], [1, NSEL]],
                          offset=4 * g * S)
                src = rap(PS16, [[8192, 128], [128, 4], [1, NSEL]],
                          offset=base16 + 64 * si)
                nc.vector.tensor_copy(out=dst, in_=src)
            if LEVEL == 40 + 5 * g + 3:  # 43->g0, 48->g1
                l43 = sb.tile([NSEL, C], F32, tag="l43")
                nc.vector.memset(l43[:, :], 0.0)
                nc.vector.tensor_copy(out=l43[:, 0:65],
                                      in_=qnT[0:NSEL, 4 * g, :])
                nc.sync.dma_start(out=out[:, :], in_=l43[:, :])
                return
            # attention: QK^T accumulated onto the preloaded mask, then exp
            attb = PB_ATT0 if g == 0 else PB_ATT1
            for j in range(4):
                h = 4 * g + j
                nc.tensor.matmul(out=PS[0:S, attb + S * j:attb + S * (j + 1)],
                                 lhsT=knT[:, h, :], rhs=qnT[:, h, :],
                                 start=False, stop=(j == 3))
            nc.scalar.activation(
                out=pexp[:, 4 * g:4 * (g + 1), :],
                in_=PS[0:S, attb:attb + 4 * S].rearrange(
                    "p (h s) -> p h s", h=4),
                func=AF.Exp, bias=expb_t[:, :], scale=SCALE)
            denb = 512 if g == 0 else PB_DENB
            nc.tensor.matmul(
                out=PS[0:128, denb:denb + 256],
                lhsT=ones65B[:, :],
                rhs=rap(pexp[:, :, :], [[H * S, S], [S, 4], [1, NSEL]],
                        offset=4 * g * S),
                start=True, stop=True)
            for j in range(4):
                nc.tensor.matmul(out=PS[0:C, PB_YT + NSEL * (4 * g + j):
                                         PB_YT + NSEL * (4 * g + j + 1)],
                                 lhsT=v_sb[:, 4 * g + j, :],
                                 rhs=pexp[:, 4 * g + j, 0:NSEL],
                                 start=True, stop=True)
            if LEVEL == 42 + g:
                l42 = sb.tile([NSEL, C], F32, tag="l42")
                nc.vector.memset(l42[:, :], 0.0)
                nc.vector.tensor_copy(out=l42[:, 0:64],
                                      in_=pexp[0:NSEL, 4 * g, 0:64])
                nc.sync.dma_start(out=out[:, :], in_=l42[:, :])
                return
        if LEVEL == 5:
            l5 = sb.tile([NSEL, C], F32, tag="l5")
            nc.vector.tensor_copy(out=l5[:, 0:64], in_=pexp[0:NSEL, 0, 0:64])
            nc.vector.tensor_copy(out=l5[:, 64:128], in_=pexp[0:NSEL, 1, 0:64])
            nc.sync.dma_start(out=out[:, :], in_=l5[:, :])
            return

        # gT transposes + e_gT = exp(-g) (fp16)
        e_gT = sb.tile([C, H, NSEL], F16, tag="e_gT")
        for h in range(H):
            nc.tensor.transpose(
                out=rap(PS16, [[8192, 128], [1, NSEL]],
                        offset=PB_GT16 + NSEL * h),
                in_=g_sb[:, h, :],
                identity=identh[0:NSEL, 0:NSEL])
        nc.scalar.activation(
            out=e_gT[:, :, :],
            in_=rap(PS16, [[8192, 128], [NSEL, H], [1, NSEL]],
                    offset=PB_GT16).rearrange("p h s -> p h s"),
            func=AF.Exp, scale=-1.0)

        # yg^T = yT / ((1+e^-g)*den), per group; then output projection
        D = sb.tile([128, 512], F32, tag="D")
        ygT = sb.tile([C, H, NSEL], F16, tag="ygT")
        egT_flat = e_gT[:, :, :].rearrange("p h s -> p (h s)")
        ygT_flat = ygT[:, :, :].rearrange("p h s -> p (h s)")
        Dr = sb.tile([128, 512], F32, tag="Dr")
        out_ps = PS[0:NSEL, PB_OUT:PB_OUT + C]
        out_sb = sb.tile([NSEL, C], F32, tag="out_sb")
        for g in range(2):
            cs = slice(256 * g, 256 * (g + 1))
            denb = 512 if g == 0 else PB_DENB
            nc.vector.scalar_tensor_tensor(
                out=D[:, cs], in0=egT_flat[:, cs],
                scalar=1.0, in1=PS[:, denb:denb + 256],
                op0=ALU.add, op1=ALU.mult)
            nc.vector.reciprocal(out=Dr[:, cs], in_=D[:, cs])
            nc.vector.tensor_tensor(
                out=ygT_flat[:, cs],
                in0=PS[:, PB_YT + 256 * g:PB_YT + 256 * (g + 1)],
                in1=Dr[:, cs], op=ALU.mult)
            for j in range(4):
                h = 4 * g + j
                nc.tensor.matmul(out=out_ps, lhsT=ygT[:, h, :],
                                 rhs=woTb[:, h, :], start=(h == 0),
                                 stop=(h == H - 1))
        nc.vector.tensor_copy(out=out_sb[:, :], in_=out_ps)
        nc.sync.dma_start(out=out[:, :], in_=out_sb[:, :])


def make_host_constants(inputs):
    """Host-side prep of tables derived from the (full) inputs."""
    cos = np.asarray(inputs["cos"]).reshape(S, 64).astype(np.float32)
    sin = np.asarray(inputs["sin"]).reshape(S, 64).astype(np.float32)
    sink = np.asarray(inputs["sink"]).reshape(H, C).astype(np.float32)
    tao = np.asarray(inputs["tao"]).astype(np.float32)
    wq = np.asarray(inputs["W_qkvg"]).astype(np.float32)
    wo = np.asarray(inputs["W_out"]).astype(np.float32)

    pos = np.arange(64) + 1
    cos_p = cos[pos]
    sin_p = sin[pos]
    cosdup = np.tile(np.concatenate([cos_p, cos_p], axis=1), (2, 1))
    sinpm = np.tile(np.concatenate([sin_p, -sin_p], axis=1), (2, 1))
    taocol = np.concatenate([np.full((64, 1), tao[0], np.float32),
                             np.full((64, 1), tao[1], np.float32)])
    cosdup = (cosdup * taocol).astype(np.float16)
    sinpm = (sinpm * taocol).astype(np.float16)

    # additive causal mask TRANSPOSED: mask[t, s] = 0 if pos_t <= pos_s
    posf = np.where(np.arange(S) < NSEL, np.arange(S) + 1, 0)
    cmaskm = np.where(posf[:, None] <= posf[None, :], 0.0,
                      NEG_BIG).astype(np.float16)  # [t, s]

    sn = sink / np.sqrt((sink * sink).mean(axis=-1, keepdims=True) + EPS)
    sinkTq = np.ascontiguousarray((sn * tao[0]).T).astype(np.float16)
    sinkTk = np.ascontiguousarray((sn * tao[1]).T).astype(np.float16)
    sinkv = sink.reshape(1, H * C).astype(np.float16)

    # sel16'[p, r] = -16 if p == p(r); sel16'[4, r] = 16*NP + 4*T(r) + t(r)
    sel16m = np.zeros((5, NSEL), np.float32)
    for Tn in range(4):
        for p in range(4):
            for t in range(4):
                r = 16 * Tn + 4 * p + t
                sel16m[p, r] = -16.0
                sel16m[4, r] = float(16 * NP + 4 * Tn + t)

    negio = (float(NP) - np.arange(NP, dtype=np.float32)).astype(np.float16)

    wqT = np.ascontiguousarray(wq.T).astype(np.float16)
    woT = np.ascontiguousarray(
        wo.reshape(C, H, C).transpose(2, 1, 0)).astype(np.float16)

    identf32 = np.eye(128, dtype=np.float32)
    identf16 = np.eye(128, dtype=np.float16)

    def pack16(a, rows=128):
        c = a.shape[1]
        padded = np.zeros((128, c), np.float16)
        padded[:rows] = a
        return padded.view(np.float32)

    tabs2 = np.zeros((128, TB_COLS), np.float32)
    tabs2[0:5, TB_SEL16:TB_SEL16 + 64] = sel16m
    tabs2[:, TB_IDF16:TB_IDF16 + 64] = pack16(identf16)
    tabs2[:, TB_COS:TB_COS + 64] = pack16(cosdup)
    tabs2[:, TB_SIN:TB_SIN + 64] = pack16(sinpm)
    tabs2[:, TB_SINKQ:TB_SINKQ + 4] = pack16(sinkTq)
    tabs2[:, TB_SINKK:TB_SINKK + 4] = pack16(sinkTk)
    cm = np.zeros((65, 66), np.float16)
    cm[:, 0:65] = cmaskm
    tabs2[:, TB_CMASK:TB_CMASK + 33] = pack16(cm, rows=65)
    tabs2[:, TB_NEGIO:TB_NEGIO + 256] = pack16(negio.reshape(1, NP), rows=1)
    return dict(tabs2=tabs2, identd=identf32, sinkv=sinkv,
                wqT_d=wqT, woT_d=woT)


_CACHE = {}


def get_nc():
    if "nc" not in _CACHE:
        nc = bacc.Bacc("TRN2", target_bir_lowering=False, debug=False,
                       num_devices=B)
        build_kernel(nc)
        nc.compile()
        _CACHE["nc"] = nc
    return _CACHE["nc"]


def make_in_maps(inputs):
    x = np.ascontiguousarray(inputs["x"], dtype=np.float32)
    pwv = np.concatenate(
        [np.asarray(inputs["patch_w"], np.float32).ravel(),
         np.ones(128, np.float32)]).reshape(1, PATCH + 128)
    consts = make_host_constants(inputs)
    in_maps = []
    for b in range(B):
        m = {"xb": np.ascontiguousarray(x[b]), "pw": pwv}
        m.update(consts)
        in_maps.append(m)
    return in_maps


def kernel(**inputs):
    nc = get_nc()
    in_maps = make_in_maps(inputs)
    res = run_bass_kernel_spmd(nc, in_maps, core_ids=list(range(B)))
    return np.stack([r["out"] for r in res.results], axis=0)


if __name__ == "__main__":
    nc = get_nc()
    print("build ok:", len(nc.m.functions[0].allocations), "allocations")
